# revision 30
# baseline (speedup 1.0000x reference)
"""FFT-based linear convolution of two 2^23-point real signals on 8 trn2 NeuronCores.

Math: conv(a, x) = Im(ifft(fft(a + i*x)^2)) / 2, with the 2^24-point FFT done as a
3-factor (256^3) matmul FFT. Stage A (over n1) is computed r-sharded across cores,
one AllToAll reshards to k1-sharded for the middle row-FFTs (stages B, C), the
pointwise square happens in the digit-reversed domain, then the inverse stages
(C', B') run locally, a second AllToAll reshards back, and inverse stage A'
produces only the imaginary part of the first half of the time-domain signal.

v2: DMA batching (CH=1024, interleaved T1 table, plane-merged loads/stores),
middle phase processes k1 in pairs with fused [128,512] elementwise ops and
512-wide moving matmuls in stages C/B', elementwise work spread over DVE/Pool/ACT.
"""
import os
import numpy as np

os.environ.setdefault("JAX_PLATFORMS", "")
import jax

jax.config.update("jax_compilation_cache_dir", "/tmp/jax_neff_cache")
jax.config.update("jax_persistent_cache_min_entry_size_bytes", -1)
jax.config.update("jax_persistent_cache_min_compile_time_secs", 0)

import concourse.bass as bass
import concourse.tile as tile
from concourse import bacc, mybir
from concourse.bass_utils import run_bass_kernel_spmd

N = 8388608          # input length
M = 2 * N            # FFT size = 2^24
B = 256              # radix
R = B * B            # 65536
W = 8                # cores
RL = R // W          # 8192 columns of r per core
CH = 1024            # free-dim chunk in stages A / A'
NCHUNK = RL // CH    # 8
KG = 16              # middle-phase k1l pair groups (2 k1l each)
F32 = mybir.dt.float32

USE_F32R = True
MMD = mybir.dt.float32r if USE_F32R else F32
BF16 = mybir.dt.bfloat16


def build_nc():
    nc = bacc.Bacc("TRN2", target_bir_lowering=False, debug=False, num_devices=W)

    a_in = nc.dram_tensor("a_c", [128, RL], MMD, kind="ExternalInput")
    x_in = nc.dram_tensor("x_c", [128, RL], MMD, kind="ExternalInput")
    # interleaved twiddle: [256, NCHUNK, 2 (re/im), CH]
    t1c_in = nc.dram_tensor("t1c", [B, NCHUNK * 2 * CH], BF16, kind="ExternalInput")
    dr_in = nc.dram_tensor("dr", [B, B], MMD, kind="ExternalInput")
    di_in = nc.dram_tensor("di", [B, B], MMD, kind="ExternalInput")
    ndi_in = nc.dram_tensor("ndi", [B, B], MMD, kind="ExternalInput")
    t2r_in = nc.dram_tensor("t2r", [B, B], F32, kind="ExternalInput")
    t2i_in = nc.dram_tensor("t2i", [B, B], F32, kind="ExternalInput")
    aw1_in = nc.dram_tensor("aw1", [B, 128], MMD, kind="ExternalInput")
    aw2_in = nc.dram_tensor("aw2", [B, 128], MMD, kind="ExternalInput")
    y_out = nc.dram_tensor("y_c", [128, RL], F32, kind="ExternalOutput")

    rg = [list(range(W))]

    with tile.TileContext(nc) as tc:
        with tc.tile_pool(name="dram", bufs=1, space="DRAM") as dram, \
             tc.tile_pool(name="consts", bufs=1) as consts:
            cc1_in = dram.tile([W, 32, 2, RL], BF16)
            cc1_out = dram.tile([W, 32, 2, RL], BF16)
            cc2_in = dram.tile([W, 32, 2, 32, B], BF16)
            cc2_out = dram.tile([W, 32, 2, 32, B], BF16)

            # ---- constant tables in SBUF ----
            dr_row, di_row, ndi_row = [], [], []
            drb_row, dib_row, ndib_row = [], [], []  # bf16 copies (stage-B moving)
            t2r2_row, t2i2_row = [], []   # T2 halves duplicated to [128, 2B]
            for p in range(2):
                for lst, src in ((dr_row, dr_in), (di_row, di_in), (ndi_row, ndi_in)):
                    t = consts.tile([128, B], MMD, name=f"c_{src.name}_{p}", tag=f"c_{src.name}_{p}")
                    nc.sync.dma_start(t[:], src[128 * p:128 * (p + 1), :])
                    lst.append(t)
                for nm, lst, srct in (("drb", drb_row, dr_row), ("dib", dib_row, di_row),
                                      ("ndib", ndib_row, ndi_row)):
                    t = consts.tile([128, B], BF16, name=f"c_{nm}_{p}", tag=f"c_{nm}_{p}")
                    nc.scalar.copy(t[:], srct[p][:])
                    lst.append(t)

                for lst, src in ((t2r2_row, t2r_in), (t2i2_row, t2i_in)):
                    t = consts.tile([128, 2 * B], F32, name=f"c2_{src.name}_{p}", tag=f"c2_{src.name}_{p}")
                    nc.sync.dma_start(t[:, 0:B], src[128 * p:128 * (p + 1), :])
                    nc.sync.dma_start(t[:, B:2 * B], src[128 * p:128 * (p + 1), :])
                    lst.append(t)
            aw1_blk, aw2_blk = [], []
            for p in range(2):
                for lst, src in ((aw1_blk, aw1_in), (aw2_blk, aw2_in)):
                    t = consts.tile([128, 128], MMD, name=f"c_{src.name}_{p}", tag=f"c_{src.name}_{p}")
                    nc.sync.dma_start(t[:], src[128 * p:128 * (p + 1), :])
                    lst.append(t)
            # concatenated moving tables for fused [re|im] matmuls:
            # stage B (bf16): catB1=[dr|di], catB2=[ndi|dr]
            # stage C' (f32r): catC1=[dr|ndi], catC2=[di|dr]
            # twiddle cats (f32): t2ri=[t2r|t2i], t2ir=[t2i|t2r]
            catB1, catB2, catC1, catC2, t2ri, t2ir = [], [], [], [], [], []
            for p in range(2):
                for nm, lst, h0, h1 in (("catB1", catB1, drb_row, dib_row),
                                        ("catB2", catB2, ndib_row, drb_row)):
                    t = consts.tile([128, 2 * B], BF16, name=f"c_{nm}_{p}", tag=f"c_{nm}_{p}")
                    nc.scalar.copy(t[:, 0:B], h0[p][:])
                    nc.scalar.copy(t[:, B:2 * B], h1[p][:])
                    lst.append(t)
                for nm, lst, s0, s1 in (("catC1", catC1, dr_in, ndi_in),
                                        ("catC2", catC2, di_in, dr_in)):
                    t = consts.tile([128, 2 * B], MMD, name=f"c_{nm}_{p}", tag=f"c_{nm}_{p}")
                    nc.sync.dma_start(t[:, 0:B], s0[128 * p:128 * (p + 1), :])
                    nc.sync.dma_start(t[:, B:2 * B], s1[128 * p:128 * (p + 1), :])
                    lst.append(t)
                for nm, lst, s0, s1 in (("t2ri", t2ri, t2r_in, t2i_in),
                                        ("t2ir", t2ir, t2i_in, t2r_in)):
                    t = consts.tile([128, 2 * B], F32, name=f"c_{nm}_{p}", tag=f"c_{nm}_{p}")
                    nc.sync.dma_start(t[:, 0:B], s0[128 * p:128 * (p + 1), :])
                    nc.sync.dma_start(t[:, B:2 * B], s1[128 * p:128 * (p + 1), :])
                    lst.append(t)

            # ================= Phase A: stage A + T1 twiddle =================
            with tc.tile_pool(name="a_io", bufs=1) as a_io, \
                 tc.tile_pool(name="a_t1", bufs=2) as a_t1, \
                 tc.tile_pool(name="a_tmp", bufs=8) as a_tmp, \
                 tc.tile_pool(name="a_out", bufs=3) as a_outp, \
                 tc.tile_pool(name="a_ps", bufs=4, space="PSUM") as a_ps:
                a_full = a_io.tile([128, RL], MMD)
                nc.sync.dma_start(a_full[:], a_in[:, :])
                x_full = a_io.tile([128, RL], MMD)
                nc.sync.dma_start(x_full[:], x_in[:, :])

                for c in range(NCHUNK):
                    a_sl = a_full[:, c * CH:(c + 1) * CH]
                    x_sl = x_full[:, c * CH:(c + 1) * CH]
                    for h in range(2):
                        hs = slice(128 * h, 128 * (h + 1))
                        ps_r = a_ps.tile([128, CH], F32, tag="ps")
                        ps_i = a_ps.tile([128, CH], F32, tag="ps")
                        for q in range(2):
                            qs = slice(q * 512, (q + 1) * 512)
                            nc.tensor.matmul(ps_r[:, qs], dr_row[0][:, hs], a_sl[:, qs],
                                             start=True, stop=False)
                            nc.tensor.matmul(ps_i[:, qs], dr_row[0][:, hs], x_sl[:, qs],
                                             start=True, stop=False)
                            nc.tensor.matmul(ps_r[:, qs], ndi_row[0][:, hs], x_sl[:, qs],
                                             start=False, stop=True)
                            nc.tensor.matmul(ps_i[:, qs], di_row[0][:, hs], a_sl[:, qs],
                                             start=False, stop=True)

                        t1_t = a_t1.tile([128, 2 * CH], BF16, tag="t1")
                        nc.sync.dma_start(t1_t[:], t1c_in[hs, c * 2 * CH:(c + 1) * 2 * CH])
                        t1r_t = t1_t[:, 0:CH]
                        t1i_t = t1_t[:, CH:2 * CH]

                        # Y' = (ps_r + i ps_i) * (t1r + i t1i), packed [Re | Im]
                        out_t = a_outp.tile([128, 2 * CH], BF16, tag="aout")
                        m1 = a_tmp.tile([128, CH], F32, tag="tmp")
                        m2 = a_tmp.tile([128, CH], F32, tag="tmp")
                        m3 = a_tmp.tile([128, CH], F32, tag="tmp")
                        m4 = a_tmp.tile([128, CH], F32, tag="tmp")
                        nc.vector.tensor_mul(m1[:], ps_r[:], t1r_t)
                        nc.vector.tensor_mul(m2[:], ps_i[:], t1i_t)
                        nc.vector.tensor_mul(m3[:], ps_r[:], t1i_t)
                        nc.vector.tensor_mul(m4[:], ps_i[:], t1r_t)
                        nc.gpsimd.tensor_sub(out_t[:, 0:CH], m1[:], m2[:])
                        nc.gpsimd.tensor_add(out_t[:, CH:2 * CH], m3[:], m4[:])

                        # store: dims (j=4, k1l=32, plane=2, rl=CH)
                        nc.sync.dma_start(
                            cc1_in[4 * h:4 * (h + 1), :, :, c * CH:(c + 1) * CH],
                            out_t[:])

            nc.gpsimd.collective_compute(
                "AllToAll", mybir.AluOpType.bypass, replica_groups=rg,
                ins=[cc1_in.opt()], outs=[cc1_out.opt()])

            # ============ Middle: per-k1-pair row FFT + square ============
            with tc.tile_pool(name="m_in", bufs=16) as m_in, \
                 tc.tile_pool(name="m_sb", bufs=16) as m_sb, \
                 tc.tile_pool(name="m_out", bufs=6) as m_out, \
                 tc.tile_pool(name="m_ps", bufs=8, space="PSUM") as m_ps:
                for kg in range(KG):
                    # load Y[k1] as (n2, n3) per (kk, n2h, plane) — v1 layout
                    y_t = []  # [kk][n2h][plane]
                    for kk in range(2):
                        rows = []
                        for n2h in range(2):
                            row = []
                            for pl in range(2):
                                t = m_in.tile([128, B], BF16, tag="yin")
                                nc.sync.dma_start(
                                    t[:], cc1_out[4 * n2h:4 * (n2h + 1), 2 * kg + kk, pl, :])
                                row.append(t)
                            rows.append(row)
                        y_t.append(rows)

                    # stage B (data as weights, fused [zr|zi] moving) + T2 twiddle
                    zt_sb = []  # [n3h] -> (ztr, zti) fused (kk, k2) [128, 2B]
                    for n3h in range(2):
                        ztr = m_sb.tile([128, 2 * B], MMD, tag="zt")
                        zti = m_sb.tile([128, 2 * B], MMD, tag="zt")
                        for kk in range(2):
                            ks = slice(kk * B, (kk + 1) * B)
                            z_f = m_ps.tile([128, 2 * B], F32, tag="mps")
                            for n2h in range(2):
                                st = n2h == 0
                                sp = n2h == 1
                                yre = y_t[kk][n2h][0][:, 128 * n3h:128 * n3h + 128]
                                yim = y_t[kk][n2h][1][:, 128 * n3h:128 * n3h + 128]
                                nc.tensor.matmul(z_f[:], yre, catB1[n2h][:],
                                                 start=st, stop=False, skip_group_check=True)
                                nc.tensor.matmul(z_f[:], yim, catB2[n2h][:],
                                                 start=False, stop=sp, skip_group_check=True)
                            p1 = m_sb.tile([128, 2 * B], F32, tag="mtmp")
                            p2 = m_sb.tile([128, 2 * B], F32, tag="mtmp")
                            nc.vector.tensor_mul(p1[:], z_f[:], t2ri[n3h][:])
                            nc.vector.tensor_mul(p2[:], z_f[:], t2ir[n3h][:])
                            nc.gpsimd.tensor_sub(ztr[:, ks], p1[:, 0:B], p1[:, B:2 * B])
                            nc.gpsimd.tensor_add(zti[:, ks], p2[:, 0:B], p2[:, B:2 * B])
                        zt_sb.append((ztr, zti))

                    # stage C (DFT stationary, 512-wide moving): U^T (k3, (kk, k2))
                    ut_ps = []
                    for k3h in range(2):
                        ks = slice(128 * k3h, 128 * (k3h + 1))
                        ur = m_ps.tile([128, 2 * B], F32, tag="mps")
                        ui = m_ps.tile([128, 2 * B], F32, tag="mps")
                        for n3h in range(2):
                            st = n3h == 0
                            sp = n3h == 1
                            nc.tensor.matmul(ur[:], dr_row[n3h][:, ks], zt_sb[n3h][0][:],
                                             start=st, stop=False, skip_group_check=True)
                            nc.tensor.matmul(ui[:], di_row[n3h][:, ks], zt_sb[n3h][0][:],
                                             start=st, stop=False, skip_group_check=True)
                            nc.tensor.matmul(ur[:], ndi_row[n3h][:, ks], zt_sb[n3h][1][:],
                                             start=False, stop=sp, skip_group_check=True)
                            nc.tensor.matmul(ui[:], dr_row[n3h][:, ks], zt_sb[n3h][1][:],
                                             start=False, stop=sp, skip_group_check=True)
                        ut_ps.append((ur, ui))

                    # square: S = U^2 (k3, (kk, k2)) -> SBUF, fused pair
                    s_sb = []
                    for k3h in range(2):
                        ur, ui = ut_ps[k3h]
                        sr = m_sb.tile([128, 2 * B], MMD, tag="ssb")
                        si = m_sb.tile([128, 2 * B], MMD, tag="ssb")
                        uc = m_sb.tile([128, 2 * B], F32, tag="mtmp")
                        q1 = m_sb.tile([128, 2 * B], F32, tag="mtmp")
                        q2 = m_sb.tile([128, 2 * B], F32, tag="mtmp")
                        nc.scalar.copy(uc[:], ur[:])
                        nc.vector.tensor_add(q1[:], uc[:], ui[:])
                        nc.vector.tensor_sub(q2[:], uc[:], ui[:])
                        nc.vector.scalar_tensor_tensor(
                            si[:], uc[:], 2.0, ui[:],
                            mybir.AluOpType.mult, mybir.AluOpType.mult)
                        nc.gpsimd.tensor_mul(sr[:], q1[:], q2[:])
                        s_sb.append((sr, si))

                    # stage C' (data as weights, fused [z2r|z2i] moving) + conj(T2)
                    y2_sb = []  # [k2h] -> (y2r, y2i) fused (kk, n3) [128, 2B]
                    for k2h in range(2):
                        y2r = m_sb.tile([128, 2 * B], MMD, tag="y2")
                        y2i = m_sb.tile([128, 2 * B], MMD, tag="y2")
                        for kk in range(2):
                            ks = slice(kk * B, (kk + 1) * B)
                            z2_f = m_ps.tile([128, 2 * B], F32, tag="mps")
                            for k3h in range(2):
                                st = k3h == 0
                                sp = k3h == 1
                                sre = s_sb[k3h][0][:, kk * B + 128 * k2h: kk * B + 128 * k2h + 128]
                                sim = s_sb[k3h][1][:, kk * B + 128 * k2h: kk * B + 128 * k2h + 128]
                                nc.tensor.matmul(z2_f[:], sre, catC1[k3h][:],
                                                 start=st, stop=False, skip_group_check=True)
                                nc.tensor.matmul(z2_f[:], sim, catC2[k3h][:],
                                                 start=False, stop=sp, skip_group_check=True)
                            p1 = m_sb.tile([128, 2 * B], F32, tag="mtmp")
                            p2 = m_sb.tile([128, 2 * B], F32, tag="mtmp")
                            nc.vector.tensor_mul(p1[:], z2_f[:], t2ri[k2h][:])
                            nc.vector.tensor_mul(p2[:], z2_f[:], t2ir[k2h][:])
                            nc.gpsimd.tensor_add(y2r[:, ks], p1[:, 0:B], p1[:, B:2 * B])
                            nc.gpsimd.tensor_sub(y2i[:, ks], p2[:, B:2 * B], p2[:, 0:B])
                        y2_sb.append((y2r, y2i))

                    # stage B' (DFT stationary, conj D, 512-wide moving): Y' (n2, (kk, n3))
                    for n2h in range(2):
                        ns = slice(128 * n2h, 128 * (n2h + 1))
                        yr = m_ps.tile([128, 2 * B], F32, tag="mps")
                        yi = m_ps.tile([128, 2 * B], F32, tag="mps")
                        for k2h in range(2):
                            st = k2h == 0
                            sp = k2h == 1
                            nc.tensor.matmul(yr[:], dr_row[k2h][:, ns], y2_sb[k2h][0][:],
                                             start=st, stop=False, skip_group_check=True)
                            nc.tensor.matmul(yi[:], dr_row[k2h][:, ns], y2_sb[k2h][1][:],
                                             start=st, stop=False, skip_group_check=True)
                            nc.tensor.matmul(yr[:], di_row[k2h][:, ns], y2_sb[k2h][1][:],
                                             start=False, stop=sp, skip_group_check=True)
                            nc.tensor.matmul(yi[:], ndi_row[k2h][:, ns], y2_sb[k2h][0][:],
                                             start=False, stop=sp, skip_group_check=True)
                        # copy fused (kk, n3) rows to SBUF, store per (plane, kk)
                        for pl, ps in ((0, yr), (1, yi)):
                            o = m_out.tile([128, 2 * B], BF16, tag="mout")
                            nc.scalar.copy(o[:], ps[:])
                            for kk in range(2):
                                nc.sync.dma_start(
                                    cc2_in[4 * n2h:4 * (n2h + 1), 2 * kg + kk, pl, :, :],
                                    o[:, kk * B:(kk + 1) * B])

            nc.gpsimd.collective_compute(
                "AllToAll", mybir.AluOpType.bypass, replica_groups=rg,
                ins=[cc2_in.opt()], outs=[cc2_out.opt()])

            # ============ Phase A': conj(T1), inverse stage A (Im only) ============
            NL = CH // B  # n2l values per chunk
            with tc.tile_pool(name="f_in", bufs=6) as f_in, \
                 tc.tile_pool(name="f_t1", bufs=2) as f_t1, \
                 tc.tile_pool(name="f_tmp", bufs=8) as f_tmp, \
                 tc.tile_pool(name="f_out", bufs=3) as f_outp, \
                 tc.tile_pool(name="f_ps", bufs=4, space="PSUM") as f_ps:
                for c in range(NCHUNK):
                    ps_o = f_ps.tile([128, CH], F32, tag="fps")
                    for h in range(2):
                        hs = slice(128 * h, 128 * (h + 1))
                        pr = f_in.tile([128, CH], BF16, tag="pin")
                        nc.sync.dma_start(
                            pr[:], cc2_out[4 * h:4 * (h + 1), :, 0, NL * c:NL * (c + 1), :])
                        pi = f_in.tile([128, CH], BF16, tag="pin")
                        nc.sync.dma_start(
                            pi[:], cc2_out[4 * h:4 * (h + 1), :, 1, NL * c:NL * (c + 1), :])
                        t1_t = f_t1.tile([128, 2 * CH], BF16, tag="ft1")
                        nc.sync.dma_start(t1_t[:], t1c_in[hs, c * 2 * CH:(c + 1) * 2 * CH])
                        t1r_t = t1_t[:, 0:CH]
                        t1i_t = t1_t[:, CH:2 * CH]

                        # Yf = P * conj(T1)
                        yfr = f_tmp.tile([128, CH], MMD, tag="yf")
                        yfi = f_tmp.tile([128, CH], MMD, tag="yf")
                        p1 = f_tmp.tile([128, CH], F32, tag="ftmp")
                        p2 = f_tmp.tile([128, CH], F32, tag="ftmp")
                        p3 = f_tmp.tile([128, CH], F32, tag="ftmp")
                        p4 = f_tmp.tile([128, CH], F32, tag="ftmp")
                        nc.vector.tensor_mul(p1[:], pr[:], t1r_t)
                        nc.gpsimd.tensor_mul(p2[:], pi[:], t1i_t)
                        nc.vector.tensor_mul(p3[:], pi[:], t1r_t)
                        nc.gpsimd.tensor_mul(p4[:], pr[:], t1i_t)
                        nc.vector.tensor_add(yfr[:], p1[:], p2[:])
                        nc.vector.tensor_sub(yfi[:], p3[:], p4[:])

                        st = h == 0
                        sp = h == 1
                        for q in range(2):
                            qs = slice(q * 512, (q + 1) * 512)
                            nc.tensor.matmul(ps_o[:, qs], aw1_blk[h][:], yfi[:, qs],
                                             start=st, stop=False, skip_group_check=True)
                            nc.tensor.matmul(ps_o[:, qs], aw2_blk[h][:], yfr[:, qs],
                                             start=False, stop=sp, skip_group_check=True)

                    o = f_outp.tile([128, CH], F32, tag="fout")
                    nc.scalar.copy(o[:], ps_o[:])
                    nc.sync.dma_start(y_out[:, c * CH:(c + 1) * CH], o[:])

    nc.compile()
    return nc


_NC = None
_TABLES = None


def _tables():
    global _TABLES
    if _TABLES is None:
        k = np.arange(B)
        D = np.exp(-2j * np.pi * np.outer(k, k) / B)
        T2 = np.exp(-2j * np.pi * np.outer(k, k) / R)
        s = 1.0 / (2.0 * M)
        dr = np.ascontiguousarray(D.real.astype(np.float32))
        di = np.ascontiguousarray(D.imag.astype(np.float32))
        t1s = []
        for c in range(W):
            r = np.arange(c * RL, (c + 1) * RL)
            T1 = np.exp(-2j * np.pi * np.outer(k, r) / M)
            import ml_dtypes
            t1r = T1.real.astype(np.float32).reshape(B, NCHUNK, CH)
            t1i = T1.imag.astype(np.float32).reshape(B, NCHUNK, CH)
            t1c = np.empty((B, NCHUNK, 2, CH), np.float32)
            t1c[:, :, 0, :] = t1r
            t1c[:, :, 1, :] = t1i
            t1s.append(np.ascontiguousarray(
                t1c.reshape(B, NCHUNK * 2 * CH).astype(ml_dtypes.bfloat16)))
        _TABLES = dict(
            dr=dr, di=di, ndi=np.ascontiguousarray(-di),
            t2r=np.ascontiguousarray(T2.real.astype(np.float32)),
            t2i=np.ascontiguousarray(T2.imag.astype(np.float32)),
            aw1=np.ascontiguousarray((s * D.real[:, :128]).astype(np.float32)),
            aw2=np.ascontiguousarray((-s * D.imag[:, :128]).astype(np.float32)),
            t1s=t1s,
        )
    return _TABLES


def make_in_maps(a, x):
    tb = _tables()
    a3 = a.reshape(128, W, RL)
    x3 = x.reshape(128, W, RL)
    in_maps = []
    for c in range(W):
        in_maps.append(dict(
            a_c=np.ascontiguousarray(a3[:, c, :]),
            x_c=np.ascontiguousarray(x3[:, c, :]),
            t1c=tb["t1s"][c],
            dr=tb["dr"], di=tb["di"], ndi=tb["ndi"],
            t2r=tb["t2r"], t2i=tb["t2i"],
            aw1=tb["aw1"], aw2=tb["aw2"],
        ))
    return in_maps


def kernel(a, x, _want_trace=False, **_unused):
    global _NC
    a = np.asarray(a, dtype=np.float32)
    x = np.asarray(x, dtype=np.float32)
    if _NC is None:
        _NC = build_nc()
    in_maps = make_in_maps(a, x)
    res = run_bass_kernel_spmd(_NC, in_maps, core_ids=list(range(W)),
                               trace=_want_trace)
    full = np.empty((128, R), dtype=np.float32)
    for c in range(W):
        full[:, c * RL:(c + 1) * RL] = res.results[c]["y_c"]
    out = full.reshape(-1)
    if _want_trace:
        return out, res
    return out


# revision 38
# speedup vs baseline: 1.0161x; 1.0161x over previous
"""FFT-based linear convolution of two 2^23-point real signals on 8 trn2 NeuronCores.

Math: conv(a, x) = Im(ifft(fft(a + i*x)^2)) / 2, with the 2^24-point FFT done as a
3-factor (256^3) matmul FFT. Stage A (over n1) is computed r-sharded across cores,
one AllToAll reshards to k1-sharded for the middle row-FFTs (stages B, C), the
pointwise square happens in the digit-reversed domain, then the inverse stages
(C', B') run locally, a second AllToAll reshards back, and inverse stage A'
produces only the imaginary part of the first half of the time-domain signal.

v2: DMA batching (CH=1024, interleaved T1 table, plane-merged loads/stores),
middle phase processes k1 in pairs with fused [128,512] elementwise ops and
512-wide moving matmuls in stages C/B', elementwise work spread over DVE/Pool/ACT.
"""
import os
import numpy as np

os.environ.setdefault("JAX_PLATFORMS", "")
import jax

jax.config.update("jax_compilation_cache_dir", "/tmp/jax_neff_cache")
jax.config.update("jax_persistent_cache_min_entry_size_bytes", -1)
jax.config.update("jax_persistent_cache_min_compile_time_secs", 0)

import concourse.bass as bass
import concourse.tile as tile
from concourse import bacc, mybir
from concourse.bass_utils import run_bass_kernel_spmd

N = 8388608          # input length
M = 2 * N            # FFT size = 2^24
B = 256              # radix
R = B * B            # 65536
W = 8                # cores
RL = R // W          # 8192 columns of r per core
CH = 1024            # free-dim chunk in stages A / A'
NCHUNK = RL // CH    # 8
KG = 16              # middle-phase k1l pair groups (2 k1l each)
F32 = mybir.dt.float32

USE_F32R = True
MMD = mybir.dt.float32r if USE_F32R else F32
BF16 = mybir.dt.bfloat16


def build_nc():
    nc = bacc.Bacc("TRN2", target_bir_lowering=False, debug=False, num_devices=W)

    a_in = nc.dram_tensor("a_c", [128, RL], BF16, kind="ExternalInput")
    x_in = nc.dram_tensor("x_c", [128, RL], BF16, kind="ExternalInput")
    # interleaved twiddle: [256, NCHUNK, 2 (re/im), CH]
    t1c_in = nc.dram_tensor("t1c", [B, NCHUNK * 2 * CH], BF16, kind="ExternalInput")
    dr_in = nc.dram_tensor("dr", [B, B], MMD, kind="ExternalInput")
    di_in = nc.dram_tensor("di", [B, B], MMD, kind="ExternalInput")
    ndi_in = nc.dram_tensor("ndi", [B, B], MMD, kind="ExternalInput")
    t2r_in = nc.dram_tensor("t2r", [B, B], F32, kind="ExternalInput")
    t2i_in = nc.dram_tensor("t2i", [B, B], F32, kind="ExternalInput")
    aw1_in = nc.dram_tensor("aw1", [B, 128], MMD, kind="ExternalInput")
    aw2_in = nc.dram_tensor("aw2", [B, 128], MMD, kind="ExternalInput")
    y_out = nc.dram_tensor("y_c", [128, RL], F32, kind="ExternalOutput")

    rg = [list(range(W))]

    with tile.TileContext(nc) as tc:
        with tc.tile_pool(name="dram", bufs=1, space="DRAM") as dram, \
             tc.tile_pool(name="consts", bufs=1) as consts:
            cc1_in = dram.tile([W, 32, 2, RL], BF16)
            cc1_out = dram.tile([W, 32, 2, RL], BF16)
            cc2_in = dram.tile([W, 32, 2, 32, B], BF16)
            cc2_out = dram.tile([W, 32, 2, 32, B], BF16)

            # ---- constant tables in SBUF ----
            dr_row, di_row, ndi_row = [], [], []
            drb_row, dib_row, ndib_row = [], [], []  # bf16 copies (stage-B moving)

            for p in range(2):
                for lst, src in ((dr_row, dr_in), (di_row, di_in), (ndi_row, ndi_in)):
                    t = consts.tile([128, B], MMD, name=f"c_{src.name}_{p}", tag=f"c_{src.name}_{p}")
                    nc.sync.dma_start(t[:], src[128 * p:128 * (p + 1), :])
                    lst.append(t)
                for nm, lst, srct in (("drb", drb_row, dr_row), ("dib", dib_row, di_row),
                                      ("ndib", ndib_row, ndi_row)):
                    t = consts.tile([128, B], BF16, name=f"c_{nm}_{p}", tag=f"c_{nm}_{p}")
                    nc.scalar.copy(t[:], srct[p][:])
                    lst.append(t)


            aw1_blk, aw2_blk = [], []
            for p in range(2):
                for lst, src in ((aw1_blk, aw1_in), (aw2_blk, aw2_in)):
                    t = consts.tile([128, 128], MMD, name=f"c_{src.name}_{p}", tag=f"c_{src.name}_{p}")
                    nc.sync.dma_start(t[:], src[128 * p:128 * (p + 1), :])
                    lst.append(t)
            # concatenated moving tables for fused [re|im] matmuls:
            # stage B (bf16): catB1=[dr|di], catB2=[ndi|dr]
            # stage C' (f32r): catC1=[dr|ndi], catC2=[di|dr]
            # twiddle cats (f32): t2ri=[t2r|t2i], t2ir=[t2i|t2r]
            catB1, catB2, catC1, catC2, t2ri, t2ir = [], [], [], [], [], []
            for p in range(2):
                for nm, lst, h0, h1 in (("catB1", catB1, drb_row, dib_row),
                                        ("catB2", catB2, ndib_row, drb_row)):
                    t = consts.tile([128, 2 * B], BF16, name=f"c_{nm}_{p}", tag=f"c_{nm}_{p}")
                    nc.scalar.copy(t[:, 0:B], h0[p][:])
                    nc.scalar.copy(t[:, B:2 * B], h1[p][:])
                    lst.append(t)
                for nm, lst, s0, s1 in (("catC1", catC1, dr_in, ndi_in),
                                        ("catC2", catC2, di_in, dr_in)):
                    t = consts.tile([128, 2 * B], MMD, name=f"c_{nm}_{p}", tag=f"c_{nm}_{p}")
                    nc.sync.dma_start(t[:, 0:B], s0[128 * p:128 * (p + 1), :])
                    nc.sync.dma_start(t[:, B:2 * B], s1[128 * p:128 * (p + 1), :])
                    lst.append(t)
                for nm, lst, s0, s1 in (("t2ri", t2ri, t2r_in, t2i_in),
                                        ("t2ir", t2ir, t2i_in, t2r_in)):
                    t = consts.tile([128, 2 * B], F32, name=f"c_{nm}_{p}", tag=f"c_{nm}_{p}")
                    nc.sync.dma_start(t[:, 0:B], s0[128 * p:128 * (p + 1), :])
                    nc.sync.dma_start(t[:, B:2 * B], s1[128 * p:128 * (p + 1), :])
                    lst.append(t)

            # ================= Phase A: stage A + T1 twiddle =================
            with tc.tile_pool(name="a_io", bufs=1) as a_io, \
                 tc.tile_pool(name="a_t1", bufs=2) as a_t1, \
                 tc.tile_pool(name="a_tmp", bufs=8) as a_tmp, \
                 tc.tile_pool(name="a_out", bufs=3) as a_outp, \
                 tc.tile_pool(name="a_ps", bufs=4, space="PSUM") as a_ps:
                a_full = a_io.tile([128, RL], BF16)
                nc.sync.dma_start(a_full[:], a_in[:, :])
                x_full = a_io.tile([128, RL], BF16)
                nc.sync.dma_start(x_full[:], x_in[:, :])

                for c in range(NCHUNK):
                    a_sl = a_full[:, c * CH:(c + 1) * CH]
                    x_sl = x_full[:, c * CH:(c + 1) * CH]
                    for h in range(2):
                        hs = slice(128 * h, 128 * (h + 1))
                        ps_r = a_ps.tile([128, CH], F32, tag="ps")
                        ps_i = a_ps.tile([128, CH], F32, tag="ps")
                        for q in range(2):
                            qs = slice(q * 512, (q + 1) * 512)
                            nc.tensor.matmul(ps_r[:, qs], drb_row[0][:, hs], a_sl[:, qs],
                                             start=True, stop=False)
                            nc.tensor.matmul(ps_i[:, qs], drb_row[0][:, hs], x_sl[:, qs],
                                             start=True, stop=False)
                            nc.tensor.matmul(ps_r[:, qs], ndib_row[0][:, hs], x_sl[:, qs],
                                             start=False, stop=True)
                            nc.tensor.matmul(ps_i[:, qs], dib_row[0][:, hs], a_sl[:, qs],
                                             start=False, stop=True)

                        t1_t = a_t1.tile([128, 2 * CH], BF16, tag="t1")
                        nc.sync.dma_start(t1_t[:], t1c_in[hs, c * 2 * CH:(c + 1) * 2 * CH])
                        t1r_t = t1_t[:, 0:CH]
                        t1i_t = t1_t[:, CH:2 * CH]

                        # Y' = (ps_r + i ps_i) * (t1r + i t1i), packed [Re | Im]
                        out_t = a_outp.tile([128, 2 * CH], BF16, tag="aout")
                        m1 = a_tmp.tile([128, CH], F32, tag="tmp")
                        m2 = a_tmp.tile([128, CH], F32, tag="tmp")
                        m3 = a_tmp.tile([128, CH], F32, tag="tmp")
                        m4 = a_tmp.tile([128, CH], F32, tag="tmp")
                        nc.vector.tensor_mul(m1[:], ps_r[:], t1r_t)
                        nc.vector.tensor_mul(m2[:], ps_i[:], t1i_t)
                        nc.vector.tensor_mul(m3[:], ps_r[:], t1i_t)
                        nc.vector.tensor_mul(m4[:], ps_i[:], t1r_t)
                        nc.gpsimd.tensor_sub(out_t[:, 0:CH], m1[:], m2[:])
                        nc.gpsimd.tensor_add(out_t[:, CH:2 * CH], m3[:], m4[:])

                        # store: dims (j=4, k1l=32, plane=2, rl=CH)
                        nc.sync.dma_start(
                            cc1_in[4 * h:4 * (h + 1), :, :, c * CH:(c + 1) * CH],
                            out_t[:])

            nc.gpsimd.collective_compute(
                "AllToAll", mybir.AluOpType.bypass, replica_groups=rg,
                ins=[cc1_in.opt()], outs=[cc1_out.opt()])

            # ============ Middle: per-k1-pair row FFT + square ============
            with tc.tile_pool(name="m_in", bufs=16) as m_in, \
                 tc.tile_pool(name="m_sb", bufs=16) as m_sb, \
                 tc.tile_pool(name="m_out", bufs=6) as m_out, \
                 tc.tile_pool(name="m_ps", bufs=8, space="PSUM") as m_ps:
                for kg in range(KG):
                    # load Y[k1] as (n2, n3) per (kk, n2h, plane) — v1 layout
                    y_t = []  # [kk][n2h][plane]
                    for kk in range(2):
                        rows = []
                        for n2h in range(2):
                            row = []
                            for pl in range(2):
                                t = m_in.tile([128, B], BF16, tag="yin")
                                nc.sync.dma_start(
                                    t[:], cc1_out[4 * n2h:4 * (n2h + 1), 2 * kg + kk, pl, :])
                                row.append(t)
                            rows.append(row)
                        y_t.append(rows)

                    # stage B (data as weights, fused [zr|zi] moving) + T2 twiddle
                    zt_sb = []  # [n3h] -> (ztr, zti) fused (kk, k2) [128, 2B]
                    for n3h in range(2):
                        ztr = m_sb.tile([128, 2 * B], MMD, tag="zt")
                        zti = m_sb.tile([128, 2 * B], MMD, tag="zt")
                        for kk in range(2):
                            ks = slice(kk * B, (kk + 1) * B)
                            z_f = m_ps.tile([128, 2 * B], F32, tag="mps")
                            for n2h in range(2):
                                st = n2h == 0
                                sp = n2h == 1
                                yre = y_t[kk][n2h][0][:, 128 * n3h:128 * n3h + 128]
                                yim = y_t[kk][n2h][1][:, 128 * n3h:128 * n3h + 128]
                                nc.tensor.matmul(z_f[:], yre, catB1[n2h][:],
                                                 start=st, stop=False, skip_group_check=True)
                                nc.tensor.matmul(z_f[:], yim, catB2[n2h][:],
                                                 start=False, stop=sp, skip_group_check=True)
                            p1 = m_sb.tile([128, 2 * B], F32, tag="mtmp")
                            p2 = m_sb.tile([128, 2 * B], F32, tag="mtmp")
                            nc.vector.tensor_mul(p1[:], z_f[:], t2ri[n3h][:])
                            nc.vector.tensor_mul(p2[:], z_f[:], t2ir[n3h][:])
                            nc.gpsimd.tensor_sub(ztr[:, ks], p1[:, 0:B], p1[:, B:2 * B])
                            nc.gpsimd.tensor_add(zti[:, ks], p2[:, 0:B], p2[:, B:2 * B])
                        zt_sb.append((ztr, zti))

                    # stage C (DFT stationary, 512-wide moving): U^T (k3, (kk, k2))
                    ut_ps = []
                    for k3h in range(2):
                        ks = slice(128 * k3h, 128 * (k3h + 1))
                        ur = m_ps.tile([128, 2 * B], F32, tag="mps")
                        ui = m_ps.tile([128, 2 * B], F32, tag="mps")
                        for n3h in range(2):
                            st = n3h == 0
                            sp = n3h == 1
                            nc.tensor.matmul(ur[:], dr_row[n3h][:, ks], zt_sb[n3h][0][:],
                                             start=st, stop=False, skip_group_check=True)
                            nc.tensor.matmul(ui[:], di_row[n3h][:, ks], zt_sb[n3h][0][:],
                                             start=st, stop=False, skip_group_check=True)
                            nc.tensor.matmul(ur[:], ndi_row[n3h][:, ks], zt_sb[n3h][1][:],
                                             start=False, stop=sp, skip_group_check=True)
                            nc.tensor.matmul(ui[:], dr_row[n3h][:, ks], zt_sb[n3h][1][:],
                                             start=False, stop=sp, skip_group_check=True)
                        ut_ps.append((ur, ui))

                    # square: S = U^2 (k3, (kk, k2)) -> SBUF, fused pair
                    s_sb = []
                    for k3h in range(2):
                        ur, ui = ut_ps[k3h]
                        sr = m_sb.tile([128, 2 * B], MMD, tag="ssb")
                        si = m_sb.tile([128, 2 * B], MMD, tag="ssb")
                        uc = m_sb.tile([128, 2 * B], F32, tag="mtmp")
                        q1 = m_sb.tile([128, 2 * B], F32, tag="mtmp")
                        q2 = m_sb.tile([128, 2 * B], F32, tag="mtmp")
                        nc.scalar.copy(uc[:], ur[:])
                        nc.vector.tensor_add(q1[:], uc[:], ui[:])
                        nc.vector.tensor_sub(q2[:], uc[:], ui[:])
                        nc.vector.scalar_tensor_tensor(
                            si[:], uc[:], 2.0, ui[:],
                            mybir.AluOpType.mult, mybir.AluOpType.mult)
                        nc.gpsimd.tensor_mul(sr[:], q1[:], q2[:])
                        s_sb.append((sr, si))

                    # stage C' (data as weights, fused [z2r|z2i] moving) + conj(T2)
                    y2_sb = []  # [k2h] -> (y2r, y2i) fused (kk, n3) [128, 2B]
                    for k2h in range(2):
                        y2r = m_sb.tile([128, 2 * B], MMD, tag="y2")
                        y2i = m_sb.tile([128, 2 * B], MMD, tag="y2")
                        for kk in range(2):
                            ks = slice(kk * B, (kk + 1) * B)
                            z2_f = m_ps.tile([128, 2 * B], F32, tag="mps")
                            for k3h in range(2):
                                st = k3h == 0
                                sp = k3h == 1
                                sre = s_sb[k3h][0][:, kk * B + 128 * k2h: kk * B + 128 * k2h + 128]
                                sim = s_sb[k3h][1][:, kk * B + 128 * k2h: kk * B + 128 * k2h + 128]
                                nc.tensor.matmul(z2_f[:], sre, catC1[k3h][:],
                                                 start=st, stop=False, skip_group_check=True)
                                nc.tensor.matmul(z2_f[:], sim, catC2[k3h][:],
                                                 start=False, stop=sp, skip_group_check=True)
                            p1 = m_sb.tile([128, 2 * B], F32, tag="mtmp")
                            p2 = m_sb.tile([128, 2 * B], F32, tag="mtmp")
                            nc.vector.tensor_mul(p1[:], z2_f[:], t2ri[k2h][:])
                            nc.vector.tensor_mul(p2[:], z2_f[:], t2ir[k2h][:])
                            nc.gpsimd.tensor_add(y2r[:, ks], p1[:, 0:B], p1[:, B:2 * B])
                            nc.gpsimd.tensor_sub(y2i[:, ks], p2[:, B:2 * B], p2[:, 0:B])
                        y2_sb.append((y2r, y2i))

                    # stage B' (DFT stationary, conj D, 512-wide moving): Y' (n2, (kk, n3))
                    for n2h in range(2):
                        ns = slice(128 * n2h, 128 * (n2h + 1))
                        yr = m_ps.tile([128, 2 * B], F32, tag="mps")
                        yi = m_ps.tile([128, 2 * B], F32, tag="mps")
                        for k2h in range(2):
                            st = k2h == 0
                            sp = k2h == 1
                            nc.tensor.matmul(yr[:], dr_row[k2h][:, ns], y2_sb[k2h][0][:],
                                             start=st, stop=False, skip_group_check=True)
                            nc.tensor.matmul(yi[:], dr_row[k2h][:, ns], y2_sb[k2h][1][:],
                                             start=st, stop=False, skip_group_check=True)
                            nc.tensor.matmul(yr[:], di_row[k2h][:, ns], y2_sb[k2h][1][:],
                                             start=False, stop=sp, skip_group_check=True)
                            nc.tensor.matmul(yi[:], ndi_row[k2h][:, ns], y2_sb[k2h][0][:],
                                             start=False, stop=sp, skip_group_check=True)
                        # copy fused (kk, n3) rows to SBUF, store per (plane, kk)
                        for pl, ps in ((0, yr), (1, yi)):
                            o = m_out.tile([128, 2 * B], BF16, tag="mout")
                            nc.scalar.copy(o[:], ps[:])
                            for kk in range(2):
                                nc.sync.dma_start(
                                    cc2_in[4 * n2h:4 * (n2h + 1), 2 * kg + kk, pl, :, :],
                                    o[:, kk * B:(kk + 1) * B])

            nc.gpsimd.collective_compute(
                "AllToAll", mybir.AluOpType.bypass, replica_groups=rg,
                ins=[cc2_in.opt()], outs=[cc2_out.opt()])

            # ============ Phase A': conj(T1), inverse stage A (Im only) ============
            NL = CH // B  # n2l values per chunk
            with tc.tile_pool(name="f_in", bufs=6) as f_in, \
                 tc.tile_pool(name="f_t1", bufs=2) as f_t1, \
                 tc.tile_pool(name="f_tmp", bufs=8) as f_tmp, \
                 tc.tile_pool(name="f_out", bufs=3) as f_outp, \
                 tc.tile_pool(name="f_ps", bufs=4, space="PSUM") as f_ps:
                for c in range(NCHUNK):
                    ps_o = f_ps.tile([128, CH], F32, tag="fps")
                    for h in range(2):
                        hs = slice(128 * h, 128 * (h + 1))
                        pp = f_in.tile([128, 2 * CH], BF16, tag="pin")
                        nc.sync.dma_start(
                            pp[:], cc2_out[4 * h:4 * (h + 1), :, :, NL * c:NL * (c + 1), :])
                        pr = pp[:, 0:CH]
                        pi = pp[:, CH:2 * CH]
                        t1_t = f_t1.tile([128, 2 * CH], BF16, tag="ft1")
                        nc.sync.dma_start(t1_t[:], t1c_in[hs, c * 2 * CH:(c + 1) * 2 * CH])
                        t1r_t = t1_t[:, 0:CH]
                        t1i_t = t1_t[:, CH:2 * CH]

                        # Yf = P * conj(T1)
                        yfr = f_tmp.tile([128, CH], MMD, tag="yf")
                        yfi = f_tmp.tile([128, CH], MMD, tag="yf")
                        p1 = f_tmp.tile([128, CH], F32, tag="ftmp")
                        p2 = f_tmp.tile([128, CH], F32, tag="ftmp")
                        p3 = f_tmp.tile([128, CH], F32, tag="ftmp")
                        p4 = f_tmp.tile([128, CH], F32, tag="ftmp")
                        nc.vector.tensor_mul(p1[:], pr, t1r_t)
                        nc.gpsimd.tensor_mul(p2[:], pi, t1i_t)
                        nc.vector.tensor_mul(p3[:], pi, t1r_t)
                        nc.gpsimd.tensor_mul(p4[:], pr, t1i_t)
                        nc.vector.tensor_add(yfr[:], p1[:], p2[:])
                        nc.vector.tensor_sub(yfi[:], p3[:], p4[:])

                        st = h == 0
                        sp = h == 1
                        for q in range(2):
                            qs = slice(q * 512, (q + 1) * 512)
                            nc.tensor.matmul(ps_o[:, qs], aw1_blk[h][:], yfi[:, qs],
                                             start=st, stop=False, skip_group_check=True)
                            nc.tensor.matmul(ps_o[:, qs], aw2_blk[h][:], yfr[:, qs],
                                             start=False, stop=sp, skip_group_check=True)

                    o = f_outp.tile([128, CH], F32, tag="fout")
                    nc.scalar.copy(o[:], ps_o[:])
                    nc.sync.dma_start(y_out[:, c * CH:(c + 1) * CH], o[:])

    nc.compile()
    return nc


_NC = None
_TABLES = None


def _tables():
    global _TABLES
    if _TABLES is None:
        k = np.arange(B)
        D = np.exp(-2j * np.pi * np.outer(k, k) / B)
        T2 = np.exp(-2j * np.pi * np.outer(k, k) / R)
        s = 1.0 / (2.0 * M)
        dr = np.ascontiguousarray(D.real.astype(np.float32))
        di = np.ascontiguousarray(D.imag.astype(np.float32))
        t1s = []
        for c in range(W):
            r = np.arange(c * RL, (c + 1) * RL)
            T1 = np.exp(-2j * np.pi * np.outer(k, r) / M)
            import ml_dtypes
            t1r = T1.real.astype(np.float32).reshape(B, NCHUNK, CH)
            t1i = T1.imag.astype(np.float32).reshape(B, NCHUNK, CH)
            t1c = np.empty((B, NCHUNK, 2, CH), np.float32)
            t1c[:, :, 0, :] = t1r
            t1c[:, :, 1, :] = t1i
            t1s.append(np.ascontiguousarray(
                t1c.reshape(B, NCHUNK * 2 * CH).astype(ml_dtypes.bfloat16)))
        _TABLES = dict(
            dr=dr, di=di, ndi=np.ascontiguousarray(-di),
            t2r=np.ascontiguousarray(T2.real.astype(np.float32)),
            t2i=np.ascontiguousarray(T2.imag.astype(np.float32)),
            aw1=np.ascontiguousarray((s * D.real[:, :128]).astype(np.float32)),
            aw2=np.ascontiguousarray((-s * D.imag[:, :128]).astype(np.float32)),
            t1s=t1s,
        )
    return _TABLES


def make_in_maps(a, x):
    tb = _tables()
    a3 = a.reshape(128, W, RL)
    x3 = x.reshape(128, W, RL)
    import ml_dtypes
    in_maps = []
    for c in range(W):
        in_maps.append(dict(
            a_c=np.ascontiguousarray(a3[:, c, :].astype(ml_dtypes.bfloat16)),
            x_c=np.ascontiguousarray(x3[:, c, :].astype(ml_dtypes.bfloat16)),
            t1c=tb["t1s"][c],
            dr=tb["dr"], di=tb["di"], ndi=tb["ndi"],
            t2r=tb["t2r"], t2i=tb["t2i"],
            aw1=tb["aw1"], aw2=tb["aw2"],
        ))
    return in_maps


def kernel(a, x, _want_trace=False, **_unused):
    global _NC
    a = np.asarray(a, dtype=np.float32)
    x = np.asarray(x, dtype=np.float32)
    if _NC is None:
        _NC = build_nc()
    in_maps = make_in_maps(a, x)
    res = run_bass_kernel_spmd(_NC, in_maps, core_ids=list(range(W)),
                               trace=_want_trace)
    full = np.empty((128, R), dtype=np.float32)
    for c in range(W):
        full[:, c * RL:(c + 1) * RL] = res.results[c]["y_c"]
    out = full.reshape(-1)
    if _want_trace:
        return out, res
    return out


# revision 41
# speedup vs baseline: 1.0176x; 1.0015x over previous
"""FFT-based linear convolution of two 2^23-point real signals on 8 trn2 NeuronCores.

Math: conv(a, x) = Im(ifft(fft(a + i*x)^2)) / 2, with the 2^24-point FFT done as a
3-factor (256^3) matmul FFT. Stage A (over n1) is computed r-sharded across cores,
one AllToAll reshards to k1-sharded for the middle row-FFTs (stages B, C), the
pointwise square happens in the digit-reversed domain, then the inverse stages
(C', B') run locally, a second AllToAll reshards back, and inverse stage A'
produces only the imaginary part of the first half of the time-domain signal.

v2: DMA batching (CH=1024, interleaved T1 table, plane-merged loads/stores),
middle phase processes k1 in pairs with fused [128,512] elementwise ops and
512-wide moving matmuls in stages C/B', elementwise work spread over DVE/Pool/ACT.
"""
import os
import numpy as np

os.environ.setdefault("JAX_PLATFORMS", "")
import jax

jax.config.update("jax_compilation_cache_dir", "/tmp/jax_neff_cache")
jax.config.update("jax_persistent_cache_min_entry_size_bytes", -1)
jax.config.update("jax_persistent_cache_min_compile_time_secs", 0)

import concourse.bass as bass
import concourse.tile as tile
from concourse import bacc, mybir
from concourse.bass_utils import run_bass_kernel_spmd

N = 8388608          # input length
M = 2 * N            # FFT size = 2^24
B = 256              # radix
R = B * B            # 65536
W = 8                # cores
RL = R // W          # 8192 columns of r per core
CH = 1024            # free-dim chunk in stages A / A'
NCHUNK = RL // CH    # 8
KG = 16              # middle-phase k1l pair groups (2 k1l each)
F32 = mybir.dt.float32

USE_F32R = True
MMD = mybir.dt.float32r if USE_F32R else F32
BF16 = mybir.dt.bfloat16


def build_nc():
    nc = bacc.Bacc("TRN2", target_bir_lowering=False, debug=False, num_devices=W)

    a_in = nc.dram_tensor("a_c", [128, RL], BF16, kind="ExternalInput")
    x_in = nc.dram_tensor("x_c", [128, RL], BF16, kind="ExternalInput")
    # interleaved twiddle: [256, NCHUNK, 2 (re/im), CH]
    t1c_in = nc.dram_tensor("t1c", [B, NCHUNK * 2 * CH], BF16, kind="ExternalInput")
    dr_in = nc.dram_tensor("dr", [B, B], MMD, kind="ExternalInput")
    di_in = nc.dram_tensor("di", [B, B], MMD, kind="ExternalInput")
    ndi_in = nc.dram_tensor("ndi", [B, B], MMD, kind="ExternalInput")
    t2r_in = nc.dram_tensor("t2r", [B, B], F32, kind="ExternalInput")
    t2i_in = nc.dram_tensor("t2i", [B, B], F32, kind="ExternalInput")
    aw1_in = nc.dram_tensor("aw1", [B, 128], MMD, kind="ExternalInput")
    aw2_in = nc.dram_tensor("aw2", [B, 128], MMD, kind="ExternalInput")
    y_out = nc.dram_tensor("y_c", [128, RL], F32, kind="ExternalOutput")

    rg = [list(range(W))]

    with tile.TileContext(nc) as tc:
        with tc.tile_pool(name="dram", bufs=1, space="DRAM") as dram, \
             tc.tile_pool(name="consts", bufs=1) as consts:
            cc1_in = dram.tile([W, 32, 2, RL], BF16)
            cc1_out = dram.tile([W, 32, 2, RL], BF16)
            cc2_in = dram.tile([W, 32, 2, 32, B], BF16)
            cc2_out = dram.tile([W, 32, 2, 32, B], BF16)

            # ---- constant tables in SBUF ----
            dr_row, di_row, ndi_row = [], [], []
            drb_row, dib_row, ndib_row = [], [], []  # bf16 copies (stage-B moving)

            for p in range(2):
                for lst, src in ((dr_row, dr_in), (di_row, di_in), (ndi_row, ndi_in)):
                    t = consts.tile([128, B], MMD, name=f"c_{src.name}_{p}", tag=f"c_{src.name}_{p}")
                    nc.sync.dma_start(t[:], src[128 * p:128 * (p + 1), :])
                    lst.append(t)
                for nm, lst, srct in (("drb", drb_row, dr_row), ("dib", dib_row, di_row),
                                      ("ndib", ndib_row, ndi_row)):
                    t = consts.tile([128, B], BF16, name=f"c_{nm}_{p}", tag=f"c_{nm}_{p}")
                    nc.scalar.copy(t[:], srct[p][:])
                    lst.append(t)


            aw1_blk, aw2_blk = [], []
            for p in range(2):
                for lst, src in ((aw1_blk, aw1_in), (aw2_blk, aw2_in)):
                    t = consts.tile([128, 128], MMD, name=f"c_{src.name}_{p}", tag=f"c_{src.name}_{p}")
                    nc.sync.dma_start(t[:], src[128 * p:128 * (p + 1), :])
                    lst.append(t)
            # concatenated moving tables for fused [re|im] matmuls:
            # stage B (bf16): catB1=[dr|di], catB2=[ndi|dr]
            # stage C' (f32r): catC1=[dr|ndi], catC2=[di|dr]
            # twiddle cats (f32): t2ri=[t2r|t2i], t2ir=[t2i|t2r]
            catB1, catB2, catC1, catC2, t2ri, t2ir = [], [], [], [], [], []
            for p in range(2):
                for nm, lst, h0, h1 in (("catB1", catB1, drb_row, dib_row),
                                        ("catB2", catB2, ndib_row, drb_row)):
                    t = consts.tile([128, 2 * B], BF16, name=f"c_{nm}_{p}", tag=f"c_{nm}_{p}")
                    nc.scalar.copy(t[:, 0:B], h0[p][:])
                    nc.scalar.copy(t[:, B:2 * B], h1[p][:])
                    lst.append(t)
                for nm, lst, s0, s1 in (("catC1", catC1, dr_in, ndi_in),
                                        ("catC2", catC2, di_in, dr_in)):
                    t = consts.tile([128, 2 * B], MMD, name=f"c_{nm}_{p}", tag=f"c_{nm}_{p}")
                    nc.sync.dma_start(t[:, 0:B], s0[128 * p:128 * (p + 1), :])
                    nc.sync.dma_start(t[:, B:2 * B], s1[128 * p:128 * (p + 1), :])
                    lst.append(t)
                for nm, lst, s0, s1 in (("t2ri", t2ri, t2r_in, t2i_in),
                                        ("t2ir", t2ir, t2i_in, t2r_in)):
                    t = consts.tile([128, 2 * B], F32, name=f"c_{nm}_{p}", tag=f"c_{nm}_{p}")
                    nc.sync.dma_start(t[:, 0:B], s0[128 * p:128 * (p + 1), :])
                    nc.sync.dma_start(t[:, B:2 * B], s1[128 * p:128 * (p + 1), :])
                    lst.append(t)

            # ================= Phase A: stage A + T1 twiddle =================
            with tc.tile_pool(name="a_io", bufs=1) as a_io, \
                 tc.tile_pool(name="a_t1", bufs=3) as a_t1, \
                 tc.tile_pool(name="a_tmp", bufs=8) as a_tmp, \
                 tc.tile_pool(name="a_out", bufs=3) as a_outp, \
                 tc.tile_pool(name="a_ps", bufs=4, space="PSUM") as a_ps:
                a_full = a_io.tile([128, RL], BF16)
                nc.sync.dma_start(a_full[:], a_in[:, :])
                x_full = a_io.tile([128, RL], BF16)
                nc.sync.dma_start(x_full[:], x_in[:, :])

                for c in range(NCHUNK):
                    a_sl = a_full[:, c * CH:(c + 1) * CH]
                    x_sl = x_full[:, c * CH:(c + 1) * CH]
                    for h in range(2):
                        hs = slice(128 * h, 128 * (h + 1))
                        ps_r = a_ps.tile([128, CH], F32, tag="ps")
                        ps_i = a_ps.tile([128, CH], F32, tag="ps")
                        for q in range(2):
                            qs = slice(q * 512, (q + 1) * 512)
                            nc.tensor.matmul(ps_r[:, qs], drb_row[0][:, hs], a_sl[:, qs],
                                             start=True, stop=False)
                            nc.tensor.matmul(ps_i[:, qs], drb_row[0][:, hs], x_sl[:, qs],
                                             start=True, stop=False)
                            nc.tensor.matmul(ps_r[:, qs], ndib_row[0][:, hs], x_sl[:, qs],
                                             start=False, stop=True)
                            nc.tensor.matmul(ps_i[:, qs], dib_row[0][:, hs], a_sl[:, qs],
                                             start=False, stop=True)

                        t1_t = a_t1.tile([128, 2 * CH], BF16, tag="t1")
                        nc.sync.dma_start(t1_t[:], t1c_in[hs, c * 2 * CH:(c + 1) * 2 * CH])
                        t1r_t = t1_t[:, 0:CH]
                        t1i_t = t1_t[:, CH:2 * CH]

                        # Y' = (ps_r + i ps_i) * (t1r + i t1i), packed [Re | Im]
                        out_t = a_outp.tile([128, 2 * CH], BF16, tag="aout")
                        m1 = a_tmp.tile([128, CH], F32, tag="tmp")
                        m2 = a_tmp.tile([128, CH], F32, tag="tmp")
                        m3 = a_tmp.tile([128, CH], F32, tag="tmp")
                        m4 = a_tmp.tile([128, CH], F32, tag="tmp")
                        nc.vector.tensor_mul(m1[:], ps_r[:], t1r_t)
                        nc.vector.tensor_mul(m2[:], ps_i[:], t1i_t)
                        nc.vector.tensor_mul(m3[:], ps_r[:], t1i_t)
                        nc.vector.tensor_mul(m4[:], ps_i[:], t1r_t)
                        nc.gpsimd.tensor_sub(out_t[:, 0:CH], m1[:], m2[:])
                        nc.gpsimd.tensor_add(out_t[:, CH:2 * CH], m3[:], m4[:])

                        # store: dims (j=4, k1l=32, plane=2, rl=CH)
                        nc.sync.dma_start(
                            cc1_in[4 * h:4 * (h + 1), :, :, c * CH:(c + 1) * CH],
                            out_t[:])

            nc.gpsimd.collective_compute(
                "AllToAll", mybir.AluOpType.bypass, replica_groups=rg,
                ins=[cc1_in.opt()], outs=[cc1_out.opt()])

            # ============ Middle: per-k1-pair row FFT + square ============
            with tc.tile_pool(name="m_in", bufs=32) as m_in, \
                 tc.tile_pool(name="m_sb", bufs=18) as m_sb, \
                 tc.tile_pool(name="m_out", bufs=8) as m_out, \
                 tc.tile_pool(name="m_ps", bufs=8, space="PSUM") as m_ps:
                for kg in range(KG):
                    # load Y[k1] as (n2, n3) per (kk, n2h, plane) — v1 layout
                    y_t = []  # [kk][n2h][plane]
                    for kk in range(2):
                        rows = []
                        for n2h in range(2):
                            row = []
                            for pl in range(2):
                                t = m_in.tile([128, B], BF16, tag="yin")
                                nc.sync.dma_start(
                                    t[:], cc1_out[4 * n2h:4 * (n2h + 1), 2 * kg + kk, pl, :])
                                row.append(t)
                            rows.append(row)
                        y_t.append(rows)

                    # stage B (data as weights, fused [zr|zi] moving) + T2 twiddle
                    zt_sb = []  # [n3h] -> (ztr, zti) fused (kk, k2) [128, 2B]
                    for n3h in range(2):
                        ztr = m_sb.tile([128, 2 * B], MMD, tag="zt")
                        zti = m_sb.tile([128, 2 * B], MMD, tag="zt")
                        for kk in range(2):
                            ks = slice(kk * B, (kk + 1) * B)
                            z_f = m_ps.tile([128, 2 * B], F32, tag="mps")
                            for n2h in range(2):
                                st = n2h == 0
                                sp = n2h == 1
                                yre = y_t[kk][n2h][0][:, 128 * n3h:128 * n3h + 128]
                                yim = y_t[kk][n2h][1][:, 128 * n3h:128 * n3h + 128]
                                nc.tensor.matmul(z_f[:], yre, catB1[n2h][:],
                                                 start=st, stop=False, skip_group_check=True)
                                nc.tensor.matmul(z_f[:], yim, catB2[n2h][:],
                                                 start=False, stop=sp, skip_group_check=True)
                            p1 = m_sb.tile([128, 2 * B], F32, tag="mtmp")
                            p2 = m_sb.tile([128, 2 * B], F32, tag="mtmp")
                            nc.vector.tensor_mul(p1[:], z_f[:], t2ri[n3h][:])
                            nc.vector.tensor_mul(p2[:], z_f[:], t2ir[n3h][:])
                            nc.gpsimd.tensor_sub(ztr[:, ks], p1[:, 0:B], p1[:, B:2 * B])
                            nc.gpsimd.tensor_add(zti[:, ks], p2[:, 0:B], p2[:, B:2 * B])
                        zt_sb.append((ztr, zti))

                    # stage C (DFT stationary, 512-wide moving): U^T (k3, (kk, k2))
                    ut_ps = []
                    for k3h in range(2):
                        ks = slice(128 * k3h, 128 * (k3h + 1))
                        ur = m_ps.tile([128, 2 * B], F32, tag="mps")
                        ui = m_ps.tile([128, 2 * B], F32, tag="mps")
                        for n3h in range(2):
                            st = n3h == 0
                            sp = n3h == 1
                            nc.tensor.matmul(ur[:], dr_row[n3h][:, ks], zt_sb[n3h][0][:],
                                             start=st, stop=False, skip_group_check=True)
                            nc.tensor.matmul(ui[:], di_row[n3h][:, ks], zt_sb[n3h][0][:],
                                             start=st, stop=False, skip_group_check=True)
                            nc.tensor.matmul(ur[:], ndi_row[n3h][:, ks], zt_sb[n3h][1][:],
                                             start=False, stop=sp, skip_group_check=True)
                            nc.tensor.matmul(ui[:], dr_row[n3h][:, ks], zt_sb[n3h][1][:],
                                             start=False, stop=sp, skip_group_check=True)
                        ut_ps.append((ur, ui))

                    # square: S = U^2 (k3, (kk, k2)) -> SBUF, fused pair
                    s_sb = []
                    for k3h in range(2):
                        ur, ui = ut_ps[k3h]
                        sr = m_sb.tile([128, 2 * B], MMD, tag="ssb")
                        si = m_sb.tile([128, 2 * B], MMD, tag="ssb")
                        uc = m_sb.tile([128, 2 * B], F32, tag="mtmp")
                        q1 = m_sb.tile([128, 2 * B], F32, tag="mtmp")
                        q2 = m_sb.tile([128, 2 * B], F32, tag="mtmp")
                        nc.scalar.copy(uc[:], ur[:])
                        nc.vector.tensor_add(q1[:], uc[:], ui[:])
                        nc.vector.tensor_sub(q2[:], uc[:], ui[:])
                        nc.vector.scalar_tensor_tensor(
                            si[:], uc[:], 2.0, ui[:],
                            mybir.AluOpType.mult, mybir.AluOpType.mult)
                        nc.gpsimd.tensor_mul(sr[:], q1[:], q2[:])
                        s_sb.append((sr, si))

                    # stage C' (data as weights, fused [z2r|z2i] moving) + conj(T2)
                    y2_sb = []  # [k2h] -> (y2r, y2i) fused (kk, n3) [128, 2B]
                    for k2h in range(2):
                        y2r = m_sb.tile([128, 2 * B], MMD, tag="y2")
                        y2i = m_sb.tile([128, 2 * B], MMD, tag="y2")
                        for kk in range(2):
                            ks = slice(kk * B, (kk + 1) * B)
                            z2_f = m_ps.tile([128, 2 * B], F32, tag="mps")
                            for k3h in range(2):
                                st = k3h == 0
                                sp = k3h == 1
                                sre = s_sb[k3h][0][:, kk * B + 128 * k2h: kk * B + 128 * k2h + 128]
                                sim = s_sb[k3h][1][:, kk * B + 128 * k2h: kk * B + 128 * k2h + 128]
                                nc.tensor.matmul(z2_f[:], sre, catC1[k3h][:],
                                                 start=st, stop=False, skip_group_check=True)
                                nc.tensor.matmul(z2_f[:], sim, catC2[k3h][:],
                                                 start=False, stop=sp, skip_group_check=True)
                            p1 = m_sb.tile([128, 2 * B], F32, tag="mtmp")
                            p2 = m_sb.tile([128, 2 * B], F32, tag="mtmp")
                            nc.vector.tensor_mul(p1[:], z2_f[:], t2ri[k2h][:])
                            nc.vector.tensor_mul(p2[:], z2_f[:], t2ir[k2h][:])
                            nc.gpsimd.tensor_add(y2r[:, ks], p1[:, 0:B], p1[:, B:2 * B])
                            nc.gpsimd.tensor_sub(y2i[:, ks], p2[:, B:2 * B], p2[:, 0:B])
                        y2_sb.append((y2r, y2i))

                    # stage B' (DFT stationary, conj D, 512-wide moving): Y' (n2, (kk, n3))
                    for n2h in range(2):
                        ns = slice(128 * n2h, 128 * (n2h + 1))
                        yr = m_ps.tile([128, 2 * B], F32, tag="mps")
                        yi = m_ps.tile([128, 2 * B], F32, tag="mps")
                        for k2h in range(2):
                            st = k2h == 0
                            sp = k2h == 1
                            nc.tensor.matmul(yr[:], dr_row[k2h][:, ns], y2_sb[k2h][0][:],
                                             start=st, stop=False, skip_group_check=True)
                            nc.tensor.matmul(yi[:], dr_row[k2h][:, ns], y2_sb[k2h][1][:],
                                             start=st, stop=False, skip_group_check=True)
                            nc.tensor.matmul(yr[:], di_row[k2h][:, ns], y2_sb[k2h][1][:],
                                             start=False, stop=sp, skip_group_check=True)
                            nc.tensor.matmul(yi[:], ndi_row[k2h][:, ns], y2_sb[k2h][0][:],
                                             start=False, stop=sp, skip_group_check=True)
                        # copy fused (kk, n3) rows to SBUF, store per (plane, kk)
                        for pl, ps in ((0, yr), (1, yi)):
                            o = m_out.tile([128, 2 * B], BF16, tag="mout")
                            nc.scalar.copy(o[:], ps[:])
                            for kk in range(2):
                                nc.sync.dma_start(
                                    cc2_in[4 * n2h:4 * (n2h + 1), 2 * kg + kk, pl, :, :],
                                    o[:, kk * B:(kk + 1) * B])

            nc.gpsimd.collective_compute(
                "AllToAll", mybir.AluOpType.bypass, replica_groups=rg,
                ins=[cc2_in.opt()], outs=[cc2_out.opt()])

            # ============ Phase A': conj(T1), inverse stage A (Im only) ============
            NL = CH // B  # n2l values per chunk
            with tc.tile_pool(name="f_in", bufs=8) as f_in, \
                 tc.tile_pool(name="f_t1", bufs=2) as f_t1, \
                 tc.tile_pool(name="f_tmp", bufs=8) as f_tmp, \
                 tc.tile_pool(name="f_out", bufs=3) as f_outp, \
                 tc.tile_pool(name="f_ps", bufs=4, space="PSUM") as f_ps:
                for c in range(NCHUNK):
                    ps_o = f_ps.tile([128, CH], F32, tag="fps")
                    for h in range(2):
                        hs = slice(128 * h, 128 * (h + 1))
                        pp = f_in.tile([128, 2 * CH], BF16, tag="pin")
                        nc.sync.dma_start(
                            pp[:], cc2_out[4 * h:4 * (h + 1), :, :, NL * c:NL * (c + 1), :])
                        pr = pp[:, 0:CH]
                        pi = pp[:, CH:2 * CH]
                        t1_t = f_t1.tile([128, 2 * CH], BF16, tag="ft1")
                        nc.sync.dma_start(t1_t[:], t1c_in[hs, c * 2 * CH:(c + 1) * 2 * CH])
                        t1r_t = t1_t[:, 0:CH]
                        t1i_t = t1_t[:, CH:2 * CH]

                        # Yf = P * conj(T1)
                        yfr = f_tmp.tile([128, CH], MMD, tag="yf")
                        yfi = f_tmp.tile([128, CH], MMD, tag="yf")
                        p1 = f_tmp.tile([128, CH], F32, tag="ftmp")
                        p2 = f_tmp.tile([128, CH], F32, tag="ftmp")
                        p3 = f_tmp.tile([128, CH], F32, tag="ftmp")
                        p4 = f_tmp.tile([128, CH], F32, tag="ftmp")
                        nc.vector.tensor_mul(p1[:], pr, t1r_t)
                        nc.gpsimd.tensor_mul(p2[:], pi, t1i_t)
                        nc.vector.tensor_mul(p3[:], pi, t1r_t)
                        nc.gpsimd.tensor_mul(p4[:], pr, t1i_t)
                        nc.vector.tensor_add(yfr[:], p1[:], p2[:])
                        nc.vector.tensor_sub(yfi[:], p3[:], p4[:])

                        st = h == 0
                        sp = h == 1
                        for q in range(2):
                            qs = slice(q * 512, (q + 1) * 512)
                            nc.tensor.matmul(ps_o[:, qs], aw1_blk[h][:], yfi[:, qs],
                                             start=st, stop=False, skip_group_check=True)
                            nc.tensor.matmul(ps_o[:, qs], aw2_blk[h][:], yfr[:, qs],
                                             start=False, stop=sp, skip_group_check=True)

                    o = f_outp.tile([128, CH], F32, tag="fout")
                    nc.scalar.copy(o[:], ps_o[:])
                    nc.sync.dma_start(y_out[:, c * CH:(c + 1) * CH], o[:])

    nc.compile()
    return nc


_NC = None
_TABLES = None


def _tables():
    global _TABLES
    if _TABLES is None:
        k = np.arange(B)
        D = np.exp(-2j * np.pi * np.outer(k, k) / B)
        T2 = np.exp(-2j * np.pi * np.outer(k, k) / R)
        s = 1.0 / (2.0 * M)
        dr = np.ascontiguousarray(D.real.astype(np.float32))
        di = np.ascontiguousarray(D.imag.astype(np.float32))
        t1s = []
        for c in range(W):
            r = np.arange(c * RL, (c + 1) * RL)
            T1 = np.exp(-2j * np.pi * np.outer(k, r) / M)
            import ml_dtypes
            t1r = T1.real.astype(np.float32).reshape(B, NCHUNK, CH)
            t1i = T1.imag.astype(np.float32).reshape(B, NCHUNK, CH)
            t1c = np.empty((B, NCHUNK, 2, CH), np.float32)
            t1c[:, :, 0, :] = t1r
            t1c[:, :, 1, :] = t1i
            t1s.append(np.ascontiguousarray(
                t1c.reshape(B, NCHUNK * 2 * CH).astype(ml_dtypes.bfloat16)))
        _TABLES = dict(
            dr=dr, di=di, ndi=np.ascontiguousarray(-di),
            t2r=np.ascontiguousarray(T2.real.astype(np.float32)),
            t2i=np.ascontiguousarray(T2.imag.astype(np.float32)),
            aw1=np.ascontiguousarray((s * D.real[:, :128]).astype(np.float32)),
            aw2=np.ascontiguousarray((-s * D.imag[:, :128]).astype(np.float32)),
            t1s=t1s,
        )
    return _TABLES


def make_in_maps(a, x):
    tb = _tables()
    a3 = a.reshape(128, W, RL)
    x3 = x.reshape(128, W, RL)
    import ml_dtypes
    in_maps = []
    for c in range(W):
        in_maps.append(dict(
            a_c=np.ascontiguousarray(a3[:, c, :].astype(ml_dtypes.bfloat16)),
            x_c=np.ascontiguousarray(x3[:, c, :].astype(ml_dtypes.bfloat16)),
            t1c=tb["t1s"][c],
            dr=tb["dr"], di=tb["di"], ndi=tb["ndi"],
            t2r=tb["t2r"], t2i=tb["t2i"],
            aw1=tb["aw1"], aw2=tb["aw2"],
        ))
    return in_maps


def kernel(a, x, _want_trace=False, **_unused):
    global _NC
    a = np.asarray(a, dtype=np.float32)
    x = np.asarray(x, dtype=np.float32)
    if _NC is None:
        _NC = build_nc()
    in_maps = make_in_maps(a, x)
    res = run_bass_kernel_spmd(_NC, in_maps, core_ids=list(range(W)),
                               trace=_want_trace)
    full = np.empty((128, R), dtype=np.float32)
    for c in range(W):
        full[:, c * RL:(c + 1) * RL] = res.results[c]["y_c"]
    out = full.reshape(-1)
    if _want_trace:
        return out, res
    return out


# revision 43
# speedup vs baseline: 1.2350x; 1.2136x over previous
"""FFT-based linear convolution of two 2^23-point real signals on 8 trn2 NeuronCores.

Math: conv(a, x) = Im(ifft(fft(a + i*x)^2)) / 2, with the 2^24-point FFT done as a
3-factor (256^3) matmul FFT. Stage A (over n1) is computed r-sharded across cores,
one AllToAll reshards to k1-sharded for the middle row-FFTs (stages B, C), the
pointwise square happens in the digit-reversed domain, then the inverse stages
(C', B') run locally, a second AllToAll reshards back, and inverse stage A'
produces only the imaginary part of the first half of the time-domain signal.

v2: DMA batching (CH=1024, interleaved T1 table, plane-merged loads/stores),
middle phase processes k1 in pairs with fused [128,512] elementwise ops and
512-wide moving matmuls in stages C/B', elementwise work spread over DVE/Pool/ACT.
"""
import os
import numpy as np

os.environ.setdefault("JAX_PLATFORMS", "")
import jax

jax.config.update("jax_compilation_cache_dir", "/tmp/jax_neff_cache")
jax.config.update("jax_persistent_cache_min_entry_size_bytes", -1)
jax.config.update("jax_persistent_cache_min_compile_time_secs", 0)

import concourse.bass as bass
import concourse.tile as tile
from concourse import bacc, mybir
from concourse.bass_utils import run_bass_kernel_spmd

N = 8388608          # input length
M = 2 * N            # FFT size = 2^24
B = 256              # radix
R = B * B            # 65536
W = 8                # cores
RL = R // W          # 8192 columns of r per core
CH = 1024            # free-dim chunk in stages A / A'
NCHUNK = RL // CH    # 8
KG = 16              # middle-phase k1l pair groups (2 k1l each)
F32 = mybir.dt.float32

USE_F32R = True
MMD = mybir.dt.float32r if USE_F32R else F32
BF16 = mybir.dt.bfloat16


def build_nc():
    nc = bacc.Bacc("TRN2", target_bir_lowering=False, debug=False, num_devices=W)

    a_in = nc.dram_tensor("a_c", [128, RL], BF16, kind="ExternalInput")
    x_in = nc.dram_tensor("x_c", [128, RL], BF16, kind="ExternalInput")
    # interleaved twiddle: [256, NCHUNK, 2 (re/im), CH]
    t1c_in = nc.dram_tensor("t1c", [B, NCHUNK * 2 * CH], BF16, kind="ExternalInput")
    dr_in = nc.dram_tensor("dr", [B, B], MMD, kind="ExternalInput")
    di_in = nc.dram_tensor("di", [B, B], MMD, kind="ExternalInput")
    ndi_in = nc.dram_tensor("ndi", [B, B], MMD, kind="ExternalInput")
    t2r_in = nc.dram_tensor("t2r", [B, B], F32, kind="ExternalInput")
    t2i_in = nc.dram_tensor("t2i", [B, B], F32, kind="ExternalInput")
    aw1_in = nc.dram_tensor("aw1", [B, 128], MMD, kind="ExternalInput")
    aw2_in = nc.dram_tensor("aw2", [B, 128], MMD, kind="ExternalInput")
    y_out = nc.dram_tensor("y_c", [128, RL], F32, kind="ExternalOutput")

    rg = [list(range(W))]

    with tile.TileContext(nc) as tc:
        with tc.tile_pool(name="dram", bufs=1, space="DRAM") as dram, \
             tc.tile_pool(name="consts", bufs=1) as consts:
            cc1_in = dram.tile([W, 32, 2, RL], BF16)
            cc1_out = dram.tile([W, 32, 2, RL], BF16)
            cc2_in = dram.tile([W, 32, 2, 32, B], BF16)
            cc2_out = dram.tile([W, 32, 2, 32, B], BF16)

            # ---- constant tables in SBUF ----
            dr_row, di_row, ndi_row = [], [], []
            drb_row, dib_row, ndib_row = [], [], []  # bf16 copies (stage-B moving)

            for p in range(2):
                for lst, src in ((dr_row, dr_in), (di_row, di_in), (ndi_row, ndi_in)):
                    t = consts.tile([128, B], MMD, name=f"c_{src.name}_{p}", tag=f"c_{src.name}_{p}")
                    nc.sync.dma_start(t[:], src[128 * p:128 * (p + 1), :])
                    lst.append(t)
                for nm, lst, srct in (("drb", drb_row, dr_row), ("dib", dib_row, di_row),
                                      ("ndib", ndib_row, ndi_row)):
                    t = consts.tile([128, B], BF16, name=f"c_{nm}_{p}", tag=f"c_{nm}_{p}")
                    nc.scalar.copy(t[:], srct[p][:])
                    lst.append(t)


            aw1_blk, aw2_blk = [], []
            for p in range(2):
                for lst, src in ((aw1_blk, aw1_in), (aw2_blk, aw2_in)):
                    t = consts.tile([128, 128], MMD, name=f"c_{src.name}_{p}", tag=f"c_{src.name}_{p}")
                    nc.sync.dma_start(t[:], src[128 * p:128 * (p + 1), :])
                    lst.append(t)
            # concatenated moving tables for fused [re|im] matmuls:
            # stage B (bf16): catB1=[dr|di], catB2=[ndi|dr]
            # stage C' (f32r): catC1=[dr|ndi], catC2=[di|dr]
            # twiddle cats (f32): t2ri=[t2r|t2i], t2ir=[t2i|t2r]
            catB1, catB2, catC1, catC2, t2ri, t2ir = [], [], [], [], [], []
            for p in range(2):
                for nm, lst, h0, h1 in (("catB1", catB1, drb_row, dib_row),
                                        ("catB2", catB2, ndib_row, drb_row)):
                    t = consts.tile([128, 2 * B], BF16, name=f"c_{nm}_{p}", tag=f"c_{nm}_{p}")
                    nc.scalar.copy(t[:, 0:B], h0[p][:])
                    nc.scalar.copy(t[:, B:2 * B], h1[p][:])
                    lst.append(t)
                for nm, lst, s0, s1 in (("catC1", catC1, dr_in, ndi_in),
                                        ("catC2", catC2, di_in, dr_in)):
                    t = consts.tile([128, 2 * B], MMD, name=f"c_{nm}_{p}", tag=f"c_{nm}_{p}")
                    nc.sync.dma_start(t[:, 0:B], s0[128 * p:128 * (p + 1), :])
                    nc.sync.dma_start(t[:, B:2 * B], s1[128 * p:128 * (p + 1), :])
                    lst.append(t)
                for nm, lst, s0, s1 in (("t2ri", t2ri, t2r_in, t2i_in),
                                        ("t2ir", t2ir, t2i_in, t2r_in)):
                    t = consts.tile([128, 2 * B], F32, name=f"c_{nm}_{p}", tag=f"c_{nm}_{p}")
                    nc.sync.dma_start(t[:, 0:B], s0[128 * p:128 * (p + 1), :])
                    nc.sync.dma_start(t[:, B:2 * B], s1[128 * p:128 * (p + 1), :])
                    lst.append(t)

            # ================= Phase A: stage A + T1 twiddle =================
            with tc.tile_pool(name="a_io", bufs=1) as a_io, \
                 tc.tile_pool(name="a_t1", bufs=4) as a_t1, \
                 tc.tile_pool(name="a_tmp", bufs=12) as a_tmp, \
                 tc.tile_pool(name="a_out", bufs=4) as a_outp, \
                 tc.tile_pool(name="a_ps", bufs=4, space="PSUM") as a_ps:
                a_full = a_io.tile([128, RL], BF16)
                nc.sync.dma_start(a_full[:], a_in[:, :])
                x_full = a_io.tile([128, RL], BF16)
                nc.sync.dma_start(x_full[:], x_in[:, :])

                for c in range(NCHUNK):
                    a_sl = a_full[:, c * CH:(c + 1) * CH]
                    x_sl = x_full[:, c * CH:(c + 1) * CH]
                    for h in range(2):
                        hs = slice(128 * h, 128 * (h + 1))
                        ps_r = a_ps.tile([128, CH], F32, tag="ps")
                        ps_i = a_ps.tile([128, CH], F32, tag="ps")
                        for q in range(2):
                            qs = slice(q * 512, (q + 1) * 512)
                            nc.tensor.matmul(ps_r[:, qs], drb_row[0][:, hs], a_sl[:, qs],
                                             start=True, stop=False)
                            nc.tensor.matmul(ps_i[:, qs], drb_row[0][:, hs], x_sl[:, qs],
                                             start=True, stop=False)
                            nc.tensor.matmul(ps_r[:, qs], ndib_row[0][:, hs], x_sl[:, qs],
                                             start=False, stop=True)
                            nc.tensor.matmul(ps_i[:, qs], dib_row[0][:, hs], a_sl[:, qs],
                                             start=False, stop=True)

                        t1_t = a_t1.tile([128, 2 * CH], BF16, tag="t1")
                        nc.sync.dma_start(t1_t[:], t1c_in[hs, c * 2 * CH:(c + 1) * 2 * CH])
                        t1r_t = t1_t[:, 0:CH]
                        t1i_t = t1_t[:, CH:2 * CH]

                        # Y' = (ps_r + i ps_i) * (t1r + i t1i), packed [Re | Im]
                        out_t = a_outp.tile([128, 2 * CH], BF16, tag="aout")
                        m1 = a_tmp.tile([128, CH], F32, tag="tmp")
                        m2 = a_tmp.tile([128, CH], F32, tag="tmp")
                        m3 = a_tmp.tile([128, CH], F32, tag="tmp")
                        m4 = a_tmp.tile([128, CH], F32, tag="tmp")
                        nc.vector.tensor_mul(m1[:], ps_r[:], t1r_t)
                        nc.vector.tensor_mul(m2[:], ps_i[:], t1i_t)
                        nc.vector.tensor_mul(m3[:], ps_r[:], t1i_t)
                        nc.vector.tensor_mul(m4[:], ps_i[:], t1r_t)
                        nc.gpsimd.tensor_sub(out_t[:, 0:CH], m1[:], m2[:])
                        nc.gpsimd.tensor_add(out_t[:, CH:2 * CH], m3[:], m4[:])

                        # store: dims (j=4, k1l=32, plane=2, rl=CH)
                        nc.sync.dma_start(
                            cc1_in[4 * h:4 * (h + 1), :, :, c * CH:(c + 1) * CH],
                            out_t[:])

            nc.gpsimd.collective_compute(
                "AllToAll", mybir.AluOpType.bypass, replica_groups=rg,
                ins=[cc1_in.opt()], outs=[cc1_out.opt()])

            # ============ Middle: per-k1-pair row FFT + square ============
            with tc.tile_pool(name="m_in", bufs=48) as m_in, \
                 tc.tile_pool(name="m_sb", bufs=18) as m_sb, \
                 tc.tile_pool(name="m_out", bufs=8) as m_out, \
                 tc.tile_pool(name="m_ps", bufs=8, space="PSUM") as m_ps:
                for kg in range(KG):
                    # load Y[k1] as (n2, n3) per (kk, n2h, plane) — v1 layout
                    y_t = []  # [kk][n2h][plane]
                    for kk in range(2):
                        rows = []
                        for n2h in range(2):
                            row = []
                            for pl in range(2):
                                t = m_in.tile([128, B], BF16, tag="yin")
                                nc.sync.dma_start(
                                    t[:], cc1_out[4 * n2h:4 * (n2h + 1), 2 * kg + kk, pl, :])
                                row.append(t)
                            rows.append(row)
                        y_t.append(rows)

                    # stage B (data as weights, fused [zr|zi] moving) + T2 twiddle
                    zt_sb = []  # [n3h] -> (ztr, zti) fused (kk, k2) [128, 2B]
                    for n3h in range(2):
                        ztr = m_sb.tile([128, 2 * B], MMD, tag="zt")
                        zti = m_sb.tile([128, 2 * B], MMD, tag="zt")
                        for kk in range(2):
                            ks = slice(kk * B, (kk + 1) * B)
                            z_f = m_ps.tile([128, 2 * B], F32, tag="mps")
                            for n2h in range(2):
                                st = n2h == 0
                                sp = n2h == 1
                                yre = y_t[kk][n2h][0][:, 128 * n3h:128 * n3h + 128]
                                yim = y_t[kk][n2h][1][:, 128 * n3h:128 * n3h + 128]
                                nc.tensor.matmul(z_f[:], yre, catB1[n2h][:],
                                                 start=st, stop=False, skip_group_check=True)
                                nc.tensor.matmul(z_f[:], yim, catB2[n2h][:],
                                                 start=False, stop=sp, skip_group_check=True)
                            p1 = m_sb.tile([128, 2 * B], F32, tag="mtmp")
                            p2 = m_sb.tile([128, 2 * B], F32, tag="mtmp")
                            nc.vector.tensor_mul(p1[:], z_f[:], t2ri[n3h][:])
                            nc.vector.tensor_mul(p2[:], z_f[:], t2ir[n3h][:])
                            nc.gpsimd.tensor_sub(ztr[:, ks], p1[:, 0:B], p1[:, B:2 * B])
                            nc.gpsimd.tensor_add(zti[:, ks], p2[:, 0:B], p2[:, B:2 * B])
                        zt_sb.append((ztr, zti))

                    # stage C (DFT stationary, 512-wide moving): U^T (k3, (kk, k2))
                    ut_ps = []
                    for k3h in range(2):
                        ks = slice(128 * k3h, 128 * (k3h + 1))
                        ur = m_ps.tile([128, 2 * B], F32, tag="mps")
                        ui = m_ps.tile([128, 2 * B], F32, tag="mps")
                        for n3h in range(2):
                            st = n3h == 0
                            sp = n3h == 1
                            nc.tensor.matmul(ur[:], dr_row[n3h][:, ks], zt_sb[n3h][0][:],
                                             start=st, stop=False, skip_group_check=True)
                            nc.tensor.matmul(ui[:], di_row[n3h][:, ks], zt_sb[n3h][0][:],
                                             start=st, stop=False, skip_group_check=True)
                            nc.tensor.matmul(ur[:], ndi_row[n3h][:, ks], zt_sb[n3h][1][:],
                                             start=False, stop=sp, skip_group_check=True)
                            nc.tensor.matmul(ui[:], dr_row[n3h][:, ks], zt_sb[n3h][1][:],
                                             start=False, stop=sp, skip_group_check=True)
                        ut_ps.append((ur, ui))

                    # square: S = U^2 (k3, (kk, k2)) -> SBUF, fused pair
                    s_sb = []
                    for k3h in range(2):
                        ur, ui = ut_ps[k3h]
                        sr = m_sb.tile([128, 2 * B], MMD, tag="ssb")
                        si = m_sb.tile([128, 2 * B], MMD, tag="ssb")
                        uc = m_sb.tile([128, 2 * B], F32, tag="mtmp")
                        q1 = m_sb.tile([128, 2 * B], F32, tag="mtmp")
                        q2 = m_sb.tile([128, 2 * B], F32, tag="mtmp")
                        nc.scalar.copy(uc[:], ur[:])
                        nc.vector.tensor_add(q1[:], uc[:], ui[:])
                        nc.vector.tensor_sub(q2[:], uc[:], ui[:])
                        nc.vector.scalar_tensor_tensor(
                            si[:], uc[:], 2.0, ui[:],
                            mybir.AluOpType.mult, mybir.AluOpType.mult)
                        nc.gpsimd.tensor_mul(sr[:], q1[:], q2[:])
                        s_sb.append((sr, si))

                    # stage C' (data as weights, fused [z2r|z2i] moving) + conj(T2)
                    y2_sb = []  # [k2h] -> (y2r, y2i) fused (kk, n3) [128, 2B]
                    for k2h in range(2):
                        y2r = m_sb.tile([128, 2 * B], MMD, tag="y2")
                        y2i = m_sb.tile([128, 2 * B], MMD, tag="y2")
                        for kk in range(2):
                            ks = slice(kk * B, (kk + 1) * B)
                            z2_f = m_ps.tile([128, 2 * B], F32, tag="mps")
                            for k3h in range(2):
                                st = k3h == 0
                                sp = k3h == 1
                                sre = s_sb[k3h][0][:, kk * B + 128 * k2h: kk * B + 128 * k2h + 128]
                                sim = s_sb[k3h][1][:, kk * B + 128 * k2h: kk * B + 128 * k2h + 128]
                                nc.tensor.matmul(z2_f[:], sre, catC1[k3h][:],
                                                 start=st, stop=False, skip_group_check=True)
                                nc.tensor.matmul(z2_f[:], sim, catC2[k3h][:],
                                                 start=False, stop=sp, skip_group_check=True)
                            p1 = m_sb.tile([128, 2 * B], F32, tag="mtmp")
                            p2 = m_sb.tile([128, 2 * B], F32, tag="mtmp")
                            nc.vector.tensor_mul(p1[:], z2_f[:], t2ri[k2h][:])
                            nc.vector.tensor_mul(p2[:], z2_f[:], t2ir[k2h][:])
                            nc.gpsimd.tensor_add(y2r[:, ks], p1[:, 0:B], p1[:, B:2 * B])
                            nc.gpsimd.tensor_sub(y2i[:, ks], p2[:, B:2 * B], p2[:, 0:B])
                        y2_sb.append((y2r, y2i))

                    # stage B' (DFT stationary, conj D, 512-wide moving): Y' (n2, (kk, n3))
                    for n2h in range(2):
                        ns = slice(128 * n2h, 128 * (n2h + 1))
                        yr = m_ps.tile([128, 2 * B], F32, tag="mps")
                        yi = m_ps.tile([128, 2 * B], F32, tag="mps")
                        for k2h in range(2):
                            st = k2h == 0
                            sp = k2h == 1
                            nc.tensor.matmul(yr[:], dr_row[k2h][:, ns], y2_sb[k2h][0][:],
                                             start=st, stop=False, skip_group_check=True)
                            nc.tensor.matmul(yi[:], dr_row[k2h][:, ns], y2_sb[k2h][1][:],
                                             start=st, stop=False, skip_group_check=True)
                            nc.tensor.matmul(yr[:], di_row[k2h][:, ns], y2_sb[k2h][1][:],
                                             start=False, stop=sp, skip_group_check=True)
                            nc.tensor.matmul(yi[:], ndi_row[k2h][:, ns], y2_sb[k2h][0][:],
                                             start=False, stop=sp, skip_group_check=True)
                        # copy fused (kk, n3) rows to SBUF, store per (plane, kk)
                        for pl, ps in ((0, yr), (1, yi)):
                            o = m_out.tile([128, 2 * B], BF16, tag="mout")
                            nc.scalar.copy(o[:], ps[:])
                            for kk in range(2):
                                nc.sync.dma_start(
                                    cc2_in[4 * n2h:4 * (n2h + 1), 2 * kg + kk, pl, :, :],
                                    o[:, kk * B:(kk + 1) * B])

            nc.gpsimd.collective_compute(
                "AllToAll", mybir.AluOpType.bypass, replica_groups=rg,
                ins=[cc2_in.opt()], outs=[cc2_out.opt()])

            # ============ Phase A': conj(T1), inverse stage A (Im only) ============
            NL = CH // B  # n2l values per chunk
            with tc.tile_pool(name="f_in", bufs=8) as f_in, \
                 tc.tile_pool(name="f_t1", bufs=2) as f_t1, \
                 tc.tile_pool(name="f_tmp", bufs=10) as f_tmp, \
                 tc.tile_pool(name="f_out", bufs=4) as f_outp, \
                 tc.tile_pool(name="f_ps", bufs=4, space="PSUM") as f_ps:
                for c in range(NCHUNK):
                    ps_o = f_ps.tile([128, CH], F32, tag="fps")
                    for h in range(2):
                        hs = slice(128 * h, 128 * (h + 1))
                        pp = f_in.tile([128, 2 * CH], BF16, tag="pin")
                        nc.sync.dma_start(
                            pp[:], cc2_out[4 * h:4 * (h + 1), :, :, NL * c:NL * (c + 1), :])
                        pr = pp[:, 0:CH]
                        pi = pp[:, CH:2 * CH]
                        t1_t = f_t1.tile([128, 2 * CH], BF16, tag="ft1")
                        nc.sync.dma_start(t1_t[:], t1c_in[hs, c * 2 * CH:(c + 1) * 2 * CH])
                        t1r_t = t1_t[:, 0:CH]
                        t1i_t = t1_t[:, CH:2 * CH]

                        # Yf = P * conj(T1)
                        yfr = f_tmp.tile([128, CH], MMD, tag="yf")
                        yfi = f_tmp.tile([128, CH], MMD, tag="yf")
                        p1 = f_tmp.tile([128, CH], F32, tag="ftmp")
                        p2 = f_tmp.tile([128, CH], F32, tag="ftmp")
                        p3 = f_tmp.tile([128, CH], F32, tag="ftmp")
                        p4 = f_tmp.tile([128, CH], F32, tag="ftmp")
                        nc.vector.tensor_mul(p1[:], pr, t1r_t)
                        nc.gpsimd.tensor_mul(p2[:], pi, t1i_t)
                        nc.vector.tensor_mul(p3[:], pi, t1r_t)
                        nc.gpsimd.tensor_mul(p4[:], pr, t1i_t)
                        nc.vector.tensor_add(yfr[:], p1[:], p2[:])
                        nc.vector.tensor_sub(yfi[:], p3[:], p4[:])

                        st = h == 0
                        sp = h == 1
                        for q in range(2):
                            qs = slice(q * 512, (q + 1) * 512)
                            nc.tensor.matmul(ps_o[:, qs], aw1_blk[h][:], yfi[:, qs],
                                             start=st, stop=False, skip_group_check=True)
                            nc.tensor.matmul(ps_o[:, qs], aw2_blk[h][:], yfr[:, qs],
                                             start=False, stop=sp, skip_group_check=True)

                    o = f_outp.tile([128, CH], F32, tag="fout")
                    nc.scalar.copy(o[:], ps_o[:])
                    nc.sync.dma_start(y_out[:, c * CH:(c + 1) * CH], o[:])

    nc.compile()
    return nc


_NC = None
_TABLES = None


def _tables():
    global _TABLES
    if _TABLES is None:
        k = np.arange(B)
        D = np.exp(-2j * np.pi * np.outer(k, k) / B)
        T2 = np.exp(-2j * np.pi * np.outer(k, k) / R)
        s = 1.0 / (2.0 * M)
        dr = np.ascontiguousarray(D.real.astype(np.float32))
        di = np.ascontiguousarray(D.imag.astype(np.float32))
        t1s = []
        for c in range(W):
            r = np.arange(c * RL, (c + 1) * RL)
            T1 = np.exp(-2j * np.pi * np.outer(k, r) / M)
            import ml_dtypes
            t1r = T1.real.astype(np.float32).reshape(B, NCHUNK, CH)
            t1i = T1.imag.astype(np.float32).reshape(B, NCHUNK, CH)
            t1c = np.empty((B, NCHUNK, 2, CH), np.float32)
            t1c[:, :, 0, :] = t1r
            t1c[:, :, 1, :] = t1i
            t1s.append(np.ascontiguousarray(
                t1c.reshape(B, NCHUNK * 2 * CH).astype(ml_dtypes.bfloat16)))
        _TABLES = dict(
            dr=dr, di=di, ndi=np.ascontiguousarray(-di),
            t2r=np.ascontiguousarray(T2.real.astype(np.float32)),
            t2i=np.ascontiguousarray(T2.imag.astype(np.float32)),
            aw1=np.ascontiguousarray((s * D.real[:, :128]).astype(np.float32)),
            aw2=np.ascontiguousarray((-s * D.imag[:, :128]).astype(np.float32)),
            t1s=t1s,
        )
    return _TABLES


def make_in_maps(a, x):
    tb = _tables()
    a3 = a.reshape(128, W, RL)
    x3 = x.reshape(128, W, RL)
    import ml_dtypes
    in_maps = []
    for c in range(W):
        in_maps.append(dict(
            a_c=np.ascontiguousarray(a3[:, c, :].astype(ml_dtypes.bfloat16)),
            x_c=np.ascontiguousarray(x3[:, c, :].astype(ml_dtypes.bfloat16)),
            t1c=tb["t1s"][c],
            dr=tb["dr"], di=tb["di"], ndi=tb["ndi"],
            t2r=tb["t2r"], t2i=tb["t2i"],
            aw1=tb["aw1"], aw2=tb["aw2"],
        ))
    return in_maps


def kernel(a, x, _want_trace=False, **_unused):
    global _NC
    a = np.asarray(a, dtype=np.float32)
    x = np.asarray(x, dtype=np.float32)
    if _NC is None:
        _NC = build_nc()
    in_maps = make_in_maps(a, x)
    res = run_bass_kernel_spmd(_NC, in_maps, core_ids=list(range(W)),
                               trace=_want_trace)
    full = np.empty((128, R), dtype=np.float32)
    for c in range(W):
        full[:, c * RL:(c + 1) * RL] = res.results[c]["y_c"]
    out = full.reshape(-1)
    if _want_trace:
        return out, res
    return out
